# revision 3
# baseline (speedup 1.0000x reference)
"""Per-edge dot product kernel for Trainium2 (8 NeuronCores).

Computes out[e] = sum(h[src[e]] * h[dst[e]], axis=-1) for
h: [100000, 64] f32, src/dst: [1000000] int indices.

Hardware notes driving the design (bedrock image -- extended GPSIMD
ucode like dma_gather is unavailable; only core SWDGE indirect DMA):
  - indirect_dma_start on HW is a per-partition windowed fetch: the
    offset AP supplies ONE index per partition, and partition p
    receives `free_size` contiguous elements starting at
    in_.flat[idx[p] * coef].  So one instruction gathers at most 128
    arbitrary rows (one per partition).
  - Each indirect DMA costs ~2.7us of serial GPSIMD descriptor-gen
    time, so the gather instruction count dominates runtime.

Design:
  - Shard edges across 8 cores (125,000 each; padded to 125,952);
    replicate h on every core.
  - Per core: supertiles of G*128 edges.  For each supertile: load
    [128, G] src/dst index tiles, issue 2*G indirect gathers (each
    [128 ,1] idx slice -> [128, 64] f32 rows), one batched DVE multiply
    [128, G*64], one batched DVE reduce -> [128, G] dots, one store.
  - Host reshapes [ST, 128, G] device output back to edge order.
"""

import sys

import numpy as np

_TRN_REPO = "/opt/trn_rl_repo"
if _TRN_REPO not in sys.path:
    sys.path.insert(0, _TRN_REPO)

N_NODES = 100000
N_EDGES = 1000000
D = 64
N_CORES = 8
E_CORE = N_EDGES // N_CORES   # 125000

G = 8                         # gathers batched per supertile
ST = 123                      # supertiles per core
E_PAD = ST * G * 128          # 125952

_PROGRAM_CACHE = {}


def _build_program():
    import concourse.bass as bass
    import concourse.tile as tile
    from concourse import bacc, mybir

    nc = bacc.Bacc("TRN2", target_bir_lowering=False, debug=False)

    h_t = nc.dram_tensor("h_nodes", [N_NODES, D], mybir.dt.float32, kind="ExternalInput")
    si_t = nc.dram_tensor("src_idx", [ST, 128, G], mybir.dt.int32, kind="ExternalInput")
    di_t = nc.dram_tensor("dst_idx", [ST, 128, G], mybir.dt.int32, kind="ExternalInput")
    out_t = nc.dram_tensor("edot", [ST, 128, G], mybir.dt.float32, kind="ExternalOutput")

    h_ap = h_t.ap()

    with tile.TileContext(nc) as tc:
        with (
            tc.tile_pool(name="idx", bufs=2) as idx_pool,
            tc.tile_pool(name="gat", bufs=2) as gat_pool,
            tc.tile_pool(name="res", bufs=2) as res_pool,
        ):
            for st in range(ST):
                si = idx_pool.tile([128, G], mybir.dt.int32, tag="si")
                di = idx_pool.tile([128, G], mybir.dt.int32, tag="di")
                nc.sync.dma_start(out=si[:], in_=si_t.ap()[st])
                nc.sync.dma_start(out=di[:], in_=di_t.ap()[st])

                hs = gat_pool.tile([128, G * D], mybir.dt.float32, tag="hs")
                hd = gat_pool.tile([128, G * D], mybir.dt.float32, tag="hd")
                for g in range(G):
                    nc.gpsimd.indirect_dma_start(
                        out=hs[:, g * D:(g + 1) * D],
                        out_offset=None,
                        in_=h_ap,
                        in_offset=bass.IndirectOffsetOnAxis(
                            ap=si[:, g:g + 1], axis=0
                        ),
                    )
                    nc.gpsimd.indirect_dma_start(
                        out=hd[:, g * D:(g + 1) * D],
                        out_offset=None,
                        in_=h_ap,
                        in_offset=bass.IndirectOffsetOnAxis(
                            ap=di[:, g:g + 1], axis=0
                        ),
                    )

                prod = gat_pool.tile([128, G * D], mybir.dt.float32, tag="prod")
                nc.vector.tensor_mul(out=prod[:], in0=hs[:], in1=hd[:])

                dots = res_pool.tile([128, G], mybir.dt.float32, tag="dots")
                nc.vector.tensor_reduce(
                    out=dots[:],
                    in_=prod[:].rearrange("p (g d) -> p g d", d=D),
                    axis=mybir.AxisListType.X,
                    op=mybir.AluOpType.add,
                )
                nc.sync.dma_start(out=out_t.ap()[st], in_=dots[:])

    nc.compile()
    return nc


def _get_program():
    if "p" not in _PROGRAM_CACHE:
        _PROGRAM_CACHE["p"] = _build_program()
    return _PROGRAM_CACHE["p"]


def _prep_idx(v):
    """[E_CORE] int -> [ST, 128, G] int32, edge e = st*G*128 + g*128 + p
    at position [st, p, g]."""
    padded = np.zeros(E_PAD, dtype=np.int32)
    padded[:E_CORE] = v
    return np.ascontiguousarray(
        padded.reshape(ST, G, 128).transpose(0, 2, 1)
    )


def _run(h, src, dst, trace=False):
    from concourse.bass_utils import run_bass_kernel_spmd

    h = np.ascontiguousarray(np.asarray(h, dtype=np.float32))
    src = np.asarray(src)
    dst = np.asarray(dst)

    in_maps = []
    for c in range(N_CORES):
        sl = slice(c * E_CORE, (c + 1) * E_CORE)
        in_maps.append(
            {
                "h_nodes": h,
                "src_idx": _prep_idx(src[sl]),
                "dst_idx": _prep_idx(dst[sl]),
            }
        )

    nc = _get_program()
    res = run_bass_kernel_spmd(nc, in_maps, list(range(N_CORES)), trace=trace)

    parts = []
    for c in range(N_CORES):
        dots = np.asarray(res.results[c]["edot"])  # [ST, 128, G]
        edge_vals = dots.transpose(0, 2, 1).reshape(E_PAD)
        parts.append(edge_vals[:E_CORE])
    return np.concatenate(parts), res


def kernel(h, src, dst):
    out, _ = _run(h, src, dst)
    return out
